# revision 3
# baseline (speedup 1.0000x reference)
"""Multi-head self-attention TRN2 Bass kernel.

Problem: x[2, 2048, 1024], 16 heads x 64 dim, fp32.
Sharding: 8 cores = 2 batches x 4 head-groups (4 heads each).
Each core computes its batch's partial output  (its 4 heads through
QKV -> attention -> output projection rows); host sums the 4 partials
per batch and adds bo.

Per-core layout strategy (avoids every attention transpose):
  - x^T [1024, 2048] built on-chip via PE transposes.
  - q^T, k^T [256, 2048]  (head h at partitions (h%2)*64.. of tile h//2)
  - V' [2048, 4, 65] bf16  (per head: V columns + a ones column)
  - scores computed TRANSPOSED: S^T[k,q] = k^T.T @ q^T  (f32r matmuls)
  - exp on ACT -> A^T bf16, directly the moving operand of
    out^T[65, q] = V'^T @ A^T ; row 64 = softmax row sums (ones trick).
  - normalize with DVE using gpsimd partition_broadcast of 1/sums.
  - out_proj: per-head K=64 PSUM accumulation with Wo row slices.
"""

import os
import numpy as np

S = 2048          # sequence length per batch
H = 1024          # hidden
G = 256           # head-group width (4 heads x 64)
HD = 65           # V' columns per head (64 + ones)
NHL = 4           # heads per core
N_CORES = 8

_CACHE = {}


def _build():
    if "nc" in _CACHE:
        return _CACHE["nc"]

    import concourse.bass as bass
    import concourse.mybir as mybir
    import concourse.tile as tile
    from concourse import bacc
    from concourse.masks import make_identity

    f32 = mybir.dt.float32
    f32r = mybir.dt.float32r
    bf16 = mybir.dt.bfloat16

    nc = bacc.Bacc("TRN2", target_bir_lowering=False, debug=False,
                   num_devices=N_CORES)

    x_in = nc.dram_tensor("x", [S, H], f32, kind="ExternalInput")
    wq_in = nc.dram_tensor("wq", [H, G], f32, kind="ExternalInput")
    wk_in = nc.dram_tensor("wk", [H, G], f32, kind="ExternalInput")
    wv_in = nc.dram_tensor("wv", [H, G], f32, kind="ExternalInput")
    bq_in = nc.dram_tensor("bq", [G, 1], f32, kind="ExternalInput")
    bk_in = nc.dram_tensor("bk", [G, 1], f32, kind="ExternalInput")
    bv_in = nc.dram_tensor("bv", [G], f32, kind="ExternalInput")
    wo_in = nc.dram_tensor("wo", [NHL, 64, H], f32, kind="ExternalInput")
    out_d = nc.dram_tensor("out", [S, H], f32, kind="ExternalOutput")

    with tile.TileContext(nc) as tc:
        with tc.tile_pool(name="persist", bufs=1) as persist:
            ident = persist.tile([128, 128], f32)
            make_identity(nc, ident)

            qT = persist.tile([128, 2, S], f32)     # [qd, m, s]
            kT = persist.tile([128, 2, S], f32)
            vp = persist.tile([128, 16, NHL, HD], bf16)  # [s-part, st, h, col]
            wo_sb = persist.tile([64, NHL, H], f32)
            bq_sb = persist.tile([128, 2, 1], f32)
            bk_sb = persist.tile([128, 2, 1], f32)
            bv_bc = persist.tile([128, G], f32)

            nc.sync.dma_start(
                out=wo_sb.bitcast(f32r), in_=wo_in.ap().rearrange("h p n -> p h n").bitcast(f32r))
            nc.sync.dma_start(
                out=bq_sb, in_=bq_in.ap().rearrange("(m p) o -> p m o", p=128))
            nc.sync.dma_start(
                out=bk_sb, in_=bk_in.ap().rearrange("(m p) o -> p m o", p=128))
            # broadcast bv along partitions (stride-0 partition AP)
            bv_ap = bass.AP(tensor=bv_in, offset=0, ap=[[0, 128], [1, G]])
            nc.gpsimd.dma_start(out=bv_bc, in_=bv_ap)

            # ones columns of V'
            nc.gpsimd.memset(vp[:, :, :, 64:65], 1.0)

            # ---------------- Phase A: x^T, QKV projections ----------------
            with (
                tc.tile_pool(name="xa", bufs=2) as xa_pool,
                tc.tile_pool(name="wqkv", bufs=1) as w_pool,
                tc.tile_pool(name="xT", bufs=1) as xT_pool,
                tc.tile_pool(name="ps_a", bufs=3, space="PSUM") as ps_a,
                tc.tile_pool(name="ps_v", bufs=2, space="PSUM") as ps_v,
            ):
                wq_sb = w_pool.tile([128, 8, G], f32)
                wk_sb = w_pool.tile([128, 8, G], f32)
                wv_sb = w_pool.tile([128, 8, G], f32)
                nc.sync.dma_start(
                    out=wq_sb.bitcast(f32r), in_=wq_in.ap().rearrange("(t p) d -> p t d", p=128).bitcast(f32r))
                nc.sync.dma_start(
                    out=wk_sb.bitcast(f32r), in_=wk_in.ap().rearrange("(t p) d -> p t d", p=128).bitcast(f32r))
                nc.sync.dma_start(
                    out=wv_sb.bitcast(f32r), in_=wv_in.ap().rearrange("(t p) d -> p t d", p=128).bitcast(f32r))

                xT = xT_pool.tile([128, 8, S], f32)   # [h-part, ht, s]

                for jc in range(4):  # 4 chunks of 512 seq positions
                    xch = xa_pool.tile([128, 4, H], f32)
                    nc.sync.dma_start(
                        out=xch,
                        in_=x_in.ap()[jc * 512:(jc + 1) * 512, :]
                        .rearrange("(i p) h -> p i h", p=128))
                    sl = slice(jc * 512, (jc + 1) * 512)
                    # transpose x chunk into xT
                    for ht in range(8):
                        ps_t = ps_a.tile([128, 512], f32, tag="tr")
                        for i in range(4):
                            nc.tensor.transpose(
                                ps_t[:, i * 128:(i + 1) * 128],
                                xch[:, i, ht * 128:(ht + 1) * 128],
                                ident)
                        nc.vector.tensor_copy(xT[:, ht, sl].bitcast(f32r), ps_t)
                    # q^T, k^T for this seq chunk
                    for w_sb, b_sb, dst in ((wq_sb, bq_sb, qT), (wk_sb, bk_sb, kT)):
                        for m in range(2):
                            ps_q = ps_a.tile([128, 512], f32, tag="qk")
                            for ht in range(8):
                                nc.tensor.matmul(
                                    ps_q,
                                    lhsT=w_sb[:, ht, m * 128:(m + 1) * 128]
                                    .bitcast(f32r),
                                    rhs=xT[:, ht, sl].bitcast(f32r),
                                    start=(ht == 0), stop=(ht == 7))
                            nc.vector.tensor_scalar_add(
                                dst[:, m, sl].bitcast(f32r), ps_q, b_sb[:, m, :])
                    # V for the 4 seq tiles of this chunk
                    for i in range(4):
                        st = jc * 4 + i
                        ps_vt = ps_v.tile([128, G], f32)
                        for ht in range(8):
                            nc.tensor.matmul(
                                ps_vt,
                                lhsT=xT[:, ht, st * 128:(st + 1) * 128]
                                .bitcast(f32r),
                                rhs=wv_sb[:, ht, :].bitcast(f32r),
                                start=(ht == 0), stop=(ht == 7))
                        nc.vector.tensor_add(
                            vp[:, st, :, 0:64],
                            ps_vt.rearrange("p (h d) -> p h d", h=NHL),
                            bv_bc.rearrange("p (h d) -> p h d", h=NHL))

            # ---------------- Phase B: attention + out_proj ----------------
            with (
                tc.tile_pool(name="attnT", bufs=2) as at_pool,
                tc.tile_pool(name="outTn", bufs=2) as on_pool,
                tc.tile_pool(name="sums", bufs=4) as sums_pool,
                tc.tile_pool(name="rbc", bufs=3) as rbc_pool,
                tc.tile_pool(name="osb", bufs=2) as osb_pool,
                tc.tile_pool(name="ps_s", bufs=2, space="PSUM") as ps_s_pool,
                tc.tile_pool(name="ps_av", bufs=2, space="PSUM") as ps_av_pool,
                tc.tile_pool(name="ps_op", bufs=2, space="PSUM") as ps_op_pool,
            ):
                for qc in range(4):  # q-chunks of 512
                    qsl = slice(qc * 512, (qc + 1) * 512)
                    outTn = on_pool.tile([64, NHL, 512], f32)
                    for h in range(4):
                        pb = (h % 2) * 64       # partition base inside qT/kT
                        mt = h // 2             # qT/kT tile index
                        attnT = at_pool.tile([128, 16, 512], bf16)
                        for kg in range(8):  # groups of 2 k-tiles
                            ps_s = ps_s_pool.tile([128, 2, 512], f32)
                            for i in range(2):
                                kt = kg * 2 + i
                                nc.tensor.matmul(
                                    ps_s[:, i, :],
                                    lhsT=kT[pb:pb + 64, mt,
                                            kt * 128:(kt + 1) * 128]
                                    .bitcast(f32r),
                                    rhs=qT[pb:pb + 64, mt, qsl].bitcast(f32r),
                                    start=True, stop=True)
                            nc.scalar.activation(
                                out=attnT[:, kg * 2:kg * 2 + 2, :],
                                in_=ps_s,
                                func=mybir.ActivationFunctionType.Exp)
                        ps_av = ps_av_pool.tile([HD, 512], f32)
                        for kt in range(16):
                            nc.tensor.matmul(
                                ps_av,
                                lhsT=vp[:, kt, h, :],
                                rhs=attnT[:, kt, :],
                                start=(kt == 0), stop=(kt == 15))
                        sums = sums_pool.tile([1, 512], f32)
                        nc.vector.tensor_copy(sums, ps_av[64:65, :])
                        recip = sums_pool.tile([1, 512], f32, tag="recip")
                        nc.vector.reciprocal(recip, sums)
                        rbc = rbc_pool.tile([64, 512], f32)
                        nc.gpsimd.partition_broadcast(rbc, recip)
                        nc.vector.tensor_mul(
                            outTn[:, h, :].bitcast(f32r), ps_av[0:64, :], rbc)
                    # output projection for this q-chunk
                    for qt in range(4):
                        osb = osb_pool.tile([128, H], f32)
                        for ncx in range(2):
                            ps_op = ps_op_pool.tile([128, 512], f32)
                            for h in range(4):
                                nc.tensor.matmul(
                                    ps_op,
                                    lhsT=outTn[:, h, qt * 128:(qt + 1) * 128]
                                    .bitcast(f32r),
                                    rhs=wo_sb[:, h, ncx * 512:(ncx + 1) * 512]
                                    .bitcast(f32r),
                                    start=(h == 0), stop=(h == 3))
                            nc.vector.tensor_copy(
                                osb[:, ncx * 512:(ncx + 1) * 512], ps_op)
                        nc.sync.dma_start(
                            out=out_d.ap()[qc * 512 + qt * 128:
                                           qc * 512 + (qt + 1) * 128, :],
                            in_=osb)

    nc.compile()
    _CACHE["nc"] = nc
    return nc


def kernel(x, Wq, bq, Wk, bk, Wv, bv, Wo, bo):
    from concourse.bass_utils import run_bass_kernel_spmd

    x = np.asarray(x, dtype=np.float32)
    Wq = np.asarray(Wq, dtype=np.float32)
    bq = np.asarray(bq, dtype=np.float32)
    Wk = np.asarray(Wk, dtype=np.float32)
    bk = np.asarray(bk, dtype=np.float32)
    Wv = np.asarray(Wv, dtype=np.float32)
    bv = np.asarray(bv, dtype=np.float32)
    Wo = np.asarray(Wo, dtype=np.float32)
    bo = np.asarray(bo, dtype=np.float32)

    scale = np.float32(1.0 / 8.0)  # 1/sqrt(64)

    nc = _build()

    in_maps = []
    for core in range(N_CORES):
        b = core // 4
        g = core % 4
        cs = slice(g * G, (g + 1) * G)
        in_maps.append({
            "x": np.ascontiguousarray(x[b]),
            "wq": np.ascontiguousarray(Wq[:, cs] * scale),
            "wk": np.ascontiguousarray(Wk[:, cs]),
            "wv": np.ascontiguousarray(Wv[:, cs]),
            "bq": np.ascontiguousarray((bq[cs] * scale).reshape(G, 1)),
            "bk": np.ascontiguousarray(bk[cs].reshape(G, 1)),
            "bv": np.ascontiguousarray(bv[cs]),
            "wo": np.ascontiguousarray(Wo[cs, :].reshape(NHL, 64, H)),
        })

    res = run_bass_kernel_spmd(nc, in_maps, core_ids=list(range(N_CORES)))

    out = np.empty((2, S, H), dtype=np.float32)
    for b in range(2):
        acc = res.results[4 * b]["out"].astype(np.float32)
        for g in range(1, 4):
            acc = acc + res.results[4 * b + g]["out"]
        out[b] = acc + bo
    return out


# revision 4
# speedup vs baseline: 1.1487x; 1.1487x over previous
"""Multi-head self-attention TRN2 Bass kernel.

Problem: x[2, 2048, 1024], 16 heads x 64 dim, fp32.
Sharding: 8 cores = 2 batches x 4 head-groups (4 heads each).
Each core computes its batch's partial output (its 4 heads through
QKV -> attention -> output projection rows); host sums the 4 partials
per batch and adds bo.

Per-core layout strategy (avoids every attention transpose):
  - x^T loaded straight from HBM via DMA xbar transpose (x cast to bf16
    on host).
  - q^T, k^T [256, 2048] bf16  (head h at partitions (h%2)*64 of tile h//2)
  - V' [2048, 4, 65] bf16  (per head: V columns + a ones column)
  - scores computed TRANSPOSED: S^T[k,q] = k^T.T @ q^T  (bf16 matmuls,
    fp32 PSUM accumulate; 1/sqrt(hd) folded into Wq/bq on host)
  - exp on ACT -> A^T bf16, directly the moving operand of
    out^T[65, q] = V'^T @ A^T ; row 64 = softmax row sums (ones trick).
  - normalize with DVE using gpsimd partition_broadcast of 1/sums.
  - out_proj: per-head K=64 PSUM accumulation with Wo row slices.
"""

import numpy as np

S = 2048          # sequence length per batch
H = 1024          # hidden
G = 256           # head-group width (4 heads x 64)
HD = 65           # V' columns per head (64 + ones)
NHL = 4           # heads per core
N_CORES = 8

_CACHE = {}


def _build():
    if "nc" in _CACHE:
        return _CACHE["nc"]

    import concourse.bass as bass
    import concourse.mybir as mybir
    import concourse.tile as tile
    from concourse import bacc

    f32 = mybir.dt.float32
    bf16 = mybir.dt.bfloat16

    nc = bacc.Bacc("TRN2", target_bir_lowering=False, debug=False,
                   num_devices=N_CORES)

    x_in = nc.dram_tensor("x", [S, H], bf16, kind="ExternalInput")
    wq_in = nc.dram_tensor("wq", [H, G], bf16, kind="ExternalInput")
    wk_in = nc.dram_tensor("wk", [H, G], bf16, kind="ExternalInput")
    wv_in = nc.dram_tensor("wv", [H, G], bf16, kind="ExternalInput")
    bq_in = nc.dram_tensor("bq", [G, 1], f32, kind="ExternalInput")
    bk_in = nc.dram_tensor("bk", [G, 1], f32, kind="ExternalInput")
    bv_in = nc.dram_tensor("bv", [G], f32, kind="ExternalInput")
    wo_in = nc.dram_tensor("wo", [NHL, 64, H], bf16, kind="ExternalInput")
    out_d = nc.dram_tensor("out", [S, H], f32, kind="ExternalOutput")

    with tile.TileContext(nc) as tc:
        with tc.tile_pool(name="persist", bufs=1) as persist:
            qT = persist.tile([128, 2, S], bf16)     # [qd, m, s]
            kT = persist.tile([128, 2, S], bf16)
            vp = persist.tile([128, 16, NHL, HD], bf16)  # [s-part, st, h, col]
            wo_sb = persist.tile([64, NHL, H], bf16)
            bq_sb = persist.tile([128, 2, 1], f32)
            bk_sb = persist.tile([128, 2, 1], f32)
            bv_bc = persist.tile([128, G], f32)

            nc.sync.dma_start(
                out=wo_sb, in_=wo_in.ap().rearrange("h p n -> p h n"))
            nc.sync.dma_start(
                out=bq_sb, in_=bq_in.ap().rearrange("(m p) o -> p m o", p=128))
            nc.sync.dma_start(
                out=bk_sb, in_=bk_in.ap().rearrange("(m p) o -> p m o", p=128))
            # broadcast bv along partitions (stride-0 partition AP)
            bv_ap = bass.AP(tensor=bv_in, offset=0, ap=[[0, 128], [1, G]])
            nc.gpsimd.dma_start(out=bv_bc, in_=bv_ap)

            # ones columns of V'
            nc.gpsimd.memset(vp[:, :, :, 64:65], 1.0)

            # ---------------- Phase A: x^T, QKV projections ----------------
            with (
                tc.tile_pool(name="wqkv", bufs=1) as w_pool,
                tc.tile_pool(name="xT", bufs=1) as xT_pool,
                tc.tile_pool(name="ps_a", bufs=4, space="PSUM") as ps_a,
                tc.tile_pool(name="ps_v", bufs=2, space="PSUM") as ps_v,
            ):
                wq_sb = w_pool.tile([128, 8, G], bf16)
                wk_sb = w_pool.tile([128, 8, G], bf16)
                wv_sb = w_pool.tile([128, 8, G], bf16)
                nc.sync.dma_start(
                    out=wq_sb, in_=wq_in.ap().rearrange("(t p) d -> p t d", p=128))
                nc.sync.dma_start(
                    out=wk_sb, in_=wk_in.ap().rearrange("(t p) d -> p t d", p=128))
                nc.sync.dma_start(
                    out=wv_sb, in_=wv_in.ap().rearrange("(t p) d -> p t d", p=128))

                xT = xT_pool.tile([128, 8, S], bf16)   # [h-part, ht, s]
                for ht in range(8):
                    nc.sync.dma_start(
                        out=xT[:, ht, :],
                        in_=x_in.ap()[:, ht * 128:(ht + 1) * 128],
                        transpose=True)

                for jc in range(4):  # 4 seq chunks of 512
                    sl = slice(jc * 512, (jc + 1) * 512)
                    # q^T, k^T for this seq chunk
                    for w_sb, b_sb, dst in ((wq_sb, bq_sb, qT), (wk_sb, bk_sb, kT)):
                        for m in range(2):
                            ps_q = ps_a.tile([128, 512], f32, tag="qk")
                            for ht in range(8):
                                nc.tensor.matmul(
                                    ps_q,
                                    lhsT=w_sb[:, ht, m * 128:(m + 1) * 128],
                                    rhs=xT[:, ht, sl],
                                    start=(ht == 0), stop=(ht == 7))
                            nc.vector.tensor_scalar_add(
                                dst[:, m, sl], ps_q, b_sb[:, m, :])
                    # V for the 4 seq tiles of this chunk
                    for i in range(4):
                        st = jc * 4 + i
                        ps_vt = ps_v.tile([128, G], f32)
                        for ht in range(8):
                            nc.tensor.matmul(
                                ps_vt,
                                lhsT=xT[:, ht, st * 128:(st + 1) * 128],
                                rhs=wv_sb[:, ht, :],
                                start=(ht == 0), stop=(ht == 7))
                        nc.vector.tensor_add(
                            vp[:, st, :, 0:64],
                            ps_vt.rearrange("p (h d) -> p h d", h=NHL),
                            bv_bc.rearrange("p (h d) -> p h d", h=NHL))

            # ---------------- Phase B: attention + out_proj ----------------
            with (
                tc.tile_pool(name="attnT", bufs=2) as at_pool,
                tc.tile_pool(name="outTn", bufs=2) as on_pool,
                tc.tile_pool(name="sums", bufs=4) as sums_pool,
                tc.tile_pool(name="rbc", bufs=3) as rbc_pool,
                tc.tile_pool(name="osb", bufs=2) as osb_pool,
                tc.tile_pool(name="ps_s", bufs=2, space="PSUM") as ps_s_pool,
                tc.tile_pool(name="ps_av", bufs=2, space="PSUM") as ps_av_pool,
                tc.tile_pool(name="ps_op", bufs=2, space="PSUM") as ps_op_pool,
            ):
                for qc in range(4):  # q-chunks of 512
                    qsl = slice(qc * 512, (qc + 1) * 512)
                    outTn = on_pool.tile([64, NHL, 512], bf16)
                    for h in range(4):
                        pb = (h % 2) * 64       # partition base inside qT/kT
                        mt = h // 2             # qT/kT tile index
                        attnT = at_pool.tile([128, 16, 512], bf16)
                        for kg in range(8):  # groups of 2 k-tiles
                            ps_s = ps_s_pool.tile([128, 2, 512], f32)
                            for i in range(2):
                                kt = kg * 2 + i
                                nc.tensor.matmul(
                                    ps_s[:, i, :],
                                    lhsT=kT[pb:pb + 64, mt,
                                            kt * 128:(kt + 1) * 128],
                                    rhs=qT[pb:pb + 64, mt, qsl],
                                    start=True, stop=True)
                            nc.scalar.activation(
                                out=attnT[:, kg * 2:kg * 2 + 2, :],
                                in_=ps_s,
                                func=mybir.ActivationFunctionType.Exp)
                        ps_av = ps_av_pool.tile([HD, 512], f32)
                        for kt in range(16):
                            nc.tensor.matmul(
                                ps_av,
                                lhsT=vp[:, kt, h, :],
                                rhs=attnT[:, kt, :],
                                start=(kt == 0), stop=(kt == 15))
                        sums = sums_pool.tile([1, 512], f32)
                        nc.vector.tensor_copy(sums, ps_av[64:65, :])
                        recip = sums_pool.tile([1, 512], f32, tag="recip")
                        nc.vector.reciprocal(recip, sums)
                        rbc = rbc_pool.tile([64, 512], f32)
                        nc.gpsimd.partition_broadcast(rbc, recip)
                        nc.vector.tensor_mul(
                            outTn[:, h, :], ps_av[0:64, :], rbc)
                    # output projection for this q-chunk
                    for qt in range(4):
                        osb = osb_pool.tile([128, H], f32)
                        for ncx in range(2):
                            ps_op = ps_op_pool.tile([128, 512], f32)
                            for h in range(4):
                                nc.tensor.matmul(
                                    ps_op,
                                    lhsT=outTn[:, h, qt * 128:(qt + 1) * 128],
                                    rhs=wo_sb[:, h, ncx * 512:(ncx + 1) * 512],
                                    start=(h == 0), stop=(h == 3))
                            nc.vector.tensor_copy(
                                osb[:, ncx * 512:(ncx + 1) * 512], ps_op)
                        nc.sync.dma_start(
                            out=out_d.ap()[qc * 512 + qt * 128:
                                           qc * 512 + (qt + 1) * 128, :],
                            in_=osb)

    nc.compile()
    _CACHE["nc"] = nc
    return nc


def make_in_maps(x, Wq, bq, Wk, bk, Wv, bv, Wo):
    import ml_dtypes
    bf = ml_dtypes.bfloat16

    x = np.asarray(x, dtype=np.float32)
    Wq = np.asarray(Wq, dtype=np.float32)
    bq = np.asarray(bq, dtype=np.float32)
    Wk = np.asarray(Wk, dtype=np.float32)
    bk = np.asarray(bk, dtype=np.float32)
    Wv = np.asarray(Wv, dtype=np.float32)
    bv = np.asarray(bv, dtype=np.float32)
    Wo = np.asarray(Wo, dtype=np.float32)

    scale = np.float32(1.0 / 8.0)  # 1/sqrt(64)

    in_maps = []
    for core in range(N_CORES):
        b = core // 4
        g = core % 4
        cs = slice(g * G, (g + 1) * G)
        in_maps.append({
            "x": np.ascontiguousarray(x[b]).astype(bf),
            "wq": np.ascontiguousarray(Wq[:, cs] * scale).astype(bf),
            "wk": np.ascontiguousarray(Wk[:, cs]).astype(bf),
            "wv": np.ascontiguousarray(Wv[:, cs]).astype(bf),
            "bq": np.ascontiguousarray((bq[cs] * scale).reshape(G, 1)),
            "bk": np.ascontiguousarray(bk[cs].reshape(G, 1)),
            "bv": np.ascontiguousarray(bv[cs]),
            "wo": np.ascontiguousarray(Wo[cs, :].reshape(NHL, 64, H)).astype(bf),
        })
    return in_maps


def kernel(x, Wq, bq, Wk, bk, Wv, bv, Wo, bo):
    from concourse.bass_utils import run_bass_kernel_spmd

    bo = np.asarray(bo, dtype=np.float32)
    nc = _build()
    in_maps = make_in_maps(x, Wq, bq, Wk, bk, Wv, bv, Wo)
    res = run_bass_kernel_spmd(nc, in_maps, core_ids=list(range(N_CORES)))

    out = np.empty((2, S, H), dtype=np.float32)
    for b in range(2):
        acc = res.results[4 * b]["out"].astype(np.float32)
        for g in range(1, 4):
            acc = acc + res.results[4 * b + g]["out"]
        out[b] = acc + bo
    return out
